# revision 1
# baseline (speedup 1.0000x reference)
import numpy as np
import jax
import jax.numpy as jnp
from jax import lax
from functools import partial
from jax.sharding import Mesh, PartitionSpec as P
from jax.experimental.shard_map import shard_map

# FastKAN CNN: 3x (fastkan_conv -> BN(train) -> ReLU -> MaxPool2x2), flatten.
# Shapes hardcoded per problem spec: x [32,16,64,64] f32, dims:
#   L1: fin=144,  fout=64,  Ws [64,1152]
#   L2: fin=576,  fout=128, Ws [128,4608]
#   L3: fin=1152, fout=256, Ws [256,9216]
# Sharding: data-parallel over batch across 8 cores (4 images/core);
# BN batch stats get an exact cross-device pmean.

G = 8
DENOM = 4.0 / (G - 1)

_mesh = None
_fn = None


def _fastkan_conv(x, ln_w, ln_b, Wb, bb, Ws):
    B, C, H, W = x.shape
    grid = jnp.linspace(-2.0, 2.0, G)
    p = lax.conv_general_dilated_patches(x, (3, 3), (1, 1), ((1, 1), (1, 1)))
    p = p.transpose(0, 2, 3, 1).reshape(B * H * W, C * 9)
    mu = p.mean(-1, keepdims=True)
    var = p.var(-1, keepdims=True)
    pn = (p - mu) / jnp.sqrt(var + 1e-5) * ln_w + ln_b
    rbf = jnp.exp(-jnp.square((pn[:, :, None] - grid) / DENOM))
    spline = rbf.reshape(B * H * W, -1) @ Ws.T
    base = jax.nn.silu(p) @ Wb.T + bb
    out = spline + base
    return out.reshape(B, H, W, -1).transpose(0, 3, 1, 2)


def _bn_relu_pool_dist(h, gamma, beta):
    # BatchNorm2d training-mode stats over the GLOBAL batch: local moments
    # then pmean across the batch axis (equal shard sizes -> exact).
    m1 = lax.pmean(h.mean((0, 2, 3)), "b")
    m2 = lax.pmean(jnp.square(h).mean((0, 2, 3)), "b")
    var = m2 - jnp.square(m1)
    h = (h - m1[None, :, None, None]) / jnp.sqrt(var[None, :, None, None] + 1e-5)
    h = h * gamma[None, :, None, None] + beta[None, :, None, None]
    h = jax.nn.relu(h)
    return lax.reduce_window(h, -jnp.inf, lax.max, (1, 1, 2, 2), (1, 1, 2, 2), "VALID")


def _forward(x, ln_w1, ln_b1, Wb1, bb1, Ws1, g1, be1,
             ln_w2, ln_b2, Wb2, bb2, Ws2, g2, be2,
             ln_w3, ln_b3, Wb3, bb3, Ws3, g3, be3):
    h = _bn_relu_pool_dist(_fastkan_conv(x, ln_w1, ln_b1, Wb1, bb1, Ws1), g1, be1)
    h = _bn_relu_pool_dist(_fastkan_conv(h, ln_w2, ln_b2, Wb2, bb2, Ws2), g2, be2)
    h = _bn_relu_pool_dist(_fastkan_conv(h, ln_w3, ln_b3, Wb3, bb3, Ws3), g3, be3)
    return h.reshape(h.shape[0], -1)


def _build():
    global _mesh, _fn
    if _fn is not None:
        return
    devs = jax.devices()[:8]
    _mesh = Mesh(np.array(devs), ("b",))
    xs = P("b")
    rep = P()
    in_specs = (xs,) + (rep,) * 21
    fn = shard_map(_forward, mesh=_mesh, in_specs=in_specs, out_specs=xs,
                   check_rep=False)
    _fn = jax.jit(fn)


def kernel(**inputs):
    _build()
    order = ["x"]
    for i in (1, 2, 3):
        order += [f"ln_w{i}", f"ln_b{i}", f"Wb{i}", f"bb{i}", f"Ws{i}",
                  f"g{i}", f"be{i}"]
    args = [jnp.asarray(np.asarray(inputs[k]), dtype=jnp.float32) for k in order]
    out = _fn(*args)
    return np.asarray(jax.device_get(out), dtype=np.float32)


# revision 2
# speedup vs baseline: 3.6328x; 3.6328x over previous
import numpy as np
import jax
import jax.numpy as jnp
from jax import lax
from functools import partial
from jax.sharding import Mesh, PartitionSpec as P
from jax.experimental.shard_map import shard_map

# FastKAN CNN: 3x (fastkan_conv -> BN(train) -> ReLU -> MaxPool2x2), flatten.
# Shapes hardcoded per problem spec: x [32,16,64,64] f32, dims:
#   L1: fin=144,  fout=64,  Ws [64,1152]
#   L2: fin=576,  fout=128, Ws [128,4608]
#   L3: fin=1152, fout=256, Ws [256,9216]
# Sharding: data-parallel over batch across 8 cores (4 images/core);
# BN batch stats get an exact cross-device pmean.

G = 8
DENOM = 4.0 / (G - 1)

_mesh = None
_fn = None


def _fastkan_conv(x, ln_w, ln_b, Wb, bb, Ws):
    B, C, H, W = x.shape
    grid = jnp.linspace(-2.0, 2.0, G)
    p = lax.conv_general_dilated_patches(x, (3, 3), (1, 1), ((1, 1), (1, 1)))
    p = p.transpose(0, 2, 3, 1).reshape(B * H * W, C * 9)
    mu = p.mean(-1, keepdims=True)
    var = p.var(-1, keepdims=True)
    pn = (p - mu) / jnp.sqrt(var + 1e-5) * ln_w + ln_b
    rbf = jnp.exp(-jnp.square((pn[:, :, None] - grid) / DENOM))
    rbf16 = rbf.reshape(B * H * W, -1).astype(jnp.bfloat16)
    spline = jax.lax.dot(rbf16, Ws.T.astype(jnp.bfloat16),
                         preferred_element_type=jnp.float32)
    base = jax.lax.dot(jax.nn.silu(p).astype(jnp.bfloat16),
                       Wb.T.astype(jnp.bfloat16),
                       preferred_element_type=jnp.float32) + bb
    out = spline + base
    return out.reshape(B, H, W, -1).transpose(0, 3, 1, 2)


def _bn_relu_pool_dist(h, gamma, beta):
    # BatchNorm2d training-mode stats over the GLOBAL batch: local moments
    # then pmean across the batch axis (equal shard sizes -> exact).
    m1 = lax.pmean(h.mean((0, 2, 3)), "b")
    m2 = lax.pmean(jnp.square(h).mean((0, 2, 3)), "b")
    var = m2 - jnp.square(m1)
    h = (h - m1[None, :, None, None]) / jnp.sqrt(var[None, :, None, None] + 1e-5)
    h = h * gamma[None, :, None, None] + beta[None, :, None, None]
    h = jax.nn.relu(h)
    return lax.reduce_window(h, -jnp.inf, lax.max, (1, 1, 2, 2), (1, 1, 2, 2), "VALID")


def _forward(x, ln_w1, ln_b1, Wb1, bb1, Ws1, g1, be1,
             ln_w2, ln_b2, Wb2, bb2, Ws2, g2, be2,
             ln_w3, ln_b3, Wb3, bb3, Ws3, g3, be3):
    h = _bn_relu_pool_dist(_fastkan_conv(x, ln_w1, ln_b1, Wb1, bb1, Ws1), g1, be1)
    h = _bn_relu_pool_dist(_fastkan_conv(h, ln_w2, ln_b2, Wb2, bb2, Ws2), g2, be2)
    h = _bn_relu_pool_dist(_fastkan_conv(h, ln_w3, ln_b3, Wb3, bb3, Ws3), g3, be3)
    return h.reshape(h.shape[0], -1)


def _build():
    global _mesh, _fn
    if _fn is not None:
        return
    devs = jax.devices()[:8]
    _mesh = Mesh(np.array(devs), ("b",))
    xs = P("b")
    rep = P()
    in_specs = (xs,) + (rep,) * 21
    fn = shard_map(_forward, mesh=_mesh, in_specs=in_specs, out_specs=xs,
                   check_rep=False)
    _fn = jax.jit(fn)


def kernel(**inputs):
    _build()
    order = ["x"]
    for i in (1, 2, 3):
        order += [f"ln_w{i}", f"ln_b{i}", f"Wb{i}", f"bb{i}", f"Ws{i}",
                  f"g{i}", f"be{i}"]
    args = [jnp.asarray(np.asarray(inputs[k]), dtype=jnp.float32) for k in order]
    out = _fn(*args)
    return np.asarray(jax.device_get(out), dtype=np.float32)
